# revision 25
# baseline (speedup 1.0000x reference)
"""Trainium2 Bass kernel for causal multi-head attention.

Problem: B=4, S=2048, D=1024, H=16 (head_dim 64), fp32.
  qkv = x @ w_attn + b_attn ; causal SDPA ; out @ w_proj + b_proj

Sharding (8 cores): data-parallel over B (4) x tensor-parallel over head
halves (2). Core c handles batch b=c//2, heads [8*(c%2), 8*(c%2)+8).
Each core computes its qkv slice, its heads' attention, and a partial
output projection (its heads' rows of w_proj); the host sums the two
partials per batch. b_proj is added on even cores (odd cores get zeros).

Device dataflow (per core):
  P1: qT,kT = (w_q|k)^T @ x^T  -> [e, s] layout (e on partitions);
      v = x @ w_v (natural [s, e]); biases + 1/sqrt(hd) fused into the
      PSUM->SBUF evacuations. v is stored augmented as [v_h | ones].
  P2: per head-pair p, query block g (512 wide), key tile t (128):
      scoresT = kT.T @ qT (2 heads row-tiled), exp on ACT (no max
      subtraction: scores are O(+-6)), block-causal via narrowed moving
      ranges + a triangular multiply on diagonal tiles, then
      [outT | den] += [v | ones].T @ expT  (fp32r requires col_grp=0xf,
      so den rides along in the same M=128 matmul, replicated over
      partitions 64:128). Normalization by 1/den applies to outT only.
  P3: y_partial = aoT.T @ w_proj_slice (+ b_proj), DMA out.

All matmuls run as float32r (E8M11 weights, full PE rate at moving
dim >= 256; measured matmul norm-rel error ~1e-7 on rounded inputs).
"""

import math
import os

import numpy as np

import concourse.bass as bass
import concourse.mybir as mybir
import concourse.tile as tile
from concourse import bacc

last_exec_time_ns = None

B, S, D, H = 4, 2048, 1024, 16
HD = D // H          # 64
HPC = H // 2         # heads per core = 8
EC = HPC * HD        # per-core qkv slice width = 512
NP = 4               # head pairs per core
QB = 512             # query block width
KT = 128             # key tile
N_QB = S // QB       # 4
N_KT = S // KT       # 16
DT = D // 128        # 8 contraction tiles

F32 = mybir.dt.float32
BF16 = mybir.dt.bfloat16

_nc_cache: dict = {}


def _round_fp32r(x: np.ndarray) -> np.ndarray:
    """Round-to-nearest-even fp32 -> fp32r (E8M11: low 12 mantissa bits 0)."""
    u = np.ascontiguousarray(x, dtype=np.float32).view(np.uint32)
    u2 = (u + 0x7FF + ((u >> 12) & 1)) & np.uint32(0xFFFFF000)
    return u2.view(np.float32)


def _build(causal: bool):
    nc = bacc.Bacc("TRN2", target_bir_lowering=False)
    xT = nc.dram_tensor("xT", [D, S], BF16, kind="ExternalInput")
    wqkv = nc.dram_tensor("wqkv", [D, 3 * EC], BF16, kind="ExternalInput")
    bqk = nc.dram_tensor("bqk", [128, 2 * EC // 128], F32, kind="ExternalInput")
    bv = nc.dram_tensor("bv", [1, EC], F32, kind="ExternalInput")
    wp = nc.dram_tensor("wp", [EC, D], BF16, kind="ExternalInput")
    bp = nc.dram_tensor("bp", [1, D], F32, kind="ExternalInput")
    tri = nc.dram_tensor("tri", [128, 128], BF16, kind="ExternalInput")
    y = nc.dram_tensor("y", [S, D], F32, kind="ExternalOutput")

    n_qk_et = 2 * EC // 128   # 8 e-tiles for q+k
    scale = 1.0 / math.sqrt(HD)
    CB = 256                  # P1 s-chunk width
    NCH = S // CB

    from collections import deque

    with tile.TileContext(nc) as tc:
        with (
            tc.tile_pool(name="consts", bufs=1) as consts,
            tc.tile_pool(name="qkvp", bufs=1) as qkvp,
            tc.tile_pool(name="p1sb", bufs=2) as p1sb,
            tc.tile_pool(name="p1w", bufs=1) as p1w,
            tc.tile_pool(name="p3w", bufs=1) as p3w,
            tc.tile_pool(name="p2e", bufs=4) as p2e,
            tc.tile_pool(name="p2r", bufs=1) as p2r,
            tc.tile_pool(name="p3y", bufs=2) as p3y,
            tc.tile_pool(name="psS", bufs=2, space="PSUM") as psS,
            tc.tile_pool(name="psO", bufs=1, space="PSUM") as psO,
            tc.tile_pool(name="psC", bufs=2, space="PSUM") as psC,
        ):
            bqk_sb = consts.tile([128, n_qk_et], F32, tag="bqk_sb")
            tri_sb = consts.tile([128, 128], BF16, tag="tri_sb")
            qT = qkvp.tile([128, NP, S], BF16, tag="qT")
            kT = qkvp.tile([128, NP, S], BF16, tag="kT")
            # augmented v: per head h and key tile t, [ones | v_h] so a
            # single M=128 matmul yields both attn@v and the softmax
            # denominator (on out partitions 0:64)
            va = qkvp.tile([128, N_KT, HPC, 128], BF16, tag="va")
            aoT = qkvp.tile([128, NP, S], BF16, tag="aoT")
            bv_sb = p1w.tile([128, EC], F32, tag="bv_sb")
            w_sb = p1w.tile([128, DT, 3 * EC], BF16, tag="w_sb")
            bp_sb = p3w.tile([128, D], F32, tag="bp_sb")
            wp_sb = p3w.tile([128, EC // 128, D], BF16, tag="wp_sb")

            def x_chunk_ap(sc):
                # one 3D DMA per chunk: [128, DT, CB] from xT[d, s]:
                # elem (p, dt, c) <- xT[dt*128+p, sc*CB+c]
                a = xT.ap()
                return bass.AP(
                    tensor=a.tensor,
                    offset=a.offset + sc * CB,
                    ap=[[S, 128], [128 * S, DT], [1, CB]],
                )

            # ---- prologue DMAs: x chunks 0-1 + biases first on sync,
            # weight slabs in dt order alternating scalar/gpsimd ----
            chunk_x = {}
            for sc in (0, 1):
                chunk_x[sc] = p1sb.tile([128, DT, CB], BF16, tag="xts",
                                        name="xts")
                nc.sync.dma_start(out=chunk_x[sc], in_=x_chunk_ap(sc))
            nc.sync.dma_start(out=bqk_sb, in_=bqk.ap())
            nc.sync.dma_start(out=bv_sb, in_=bv.ap().to_broadcast([128, EC]))
            HW_ = 3 * EC // 2
            for half in range(2):
                for dt in range(DT):
                    eng = nc.scalar if dt % 2 == 0 else nc.gpsimd
                    eng.dma_start(
                        out=w_sb[:, dt, half * HW_:(half + 1) * HW_],
                        in_=wqkv.ap()[dt * 128:(dt + 1) * 128,
                                      half * HW_:(half + 1) * HW_],
                    )
            nc.gpsimd.dma_start(out=tri_sb, in_=tri.ap())
            # P3 constants prefetched behind the x chunks on sync
            nc.sync.dma_start(out=bp_sb, in_=bp.ap().to_broadcast([128, D]))
            for eo in range(EC // 128):
                nc.sync.dma_start(
                    out=wp_sb[:, eo, :],
                    in_=wp.ap()[eo * 128:(eo + 1) * 128, :],
                )
            # warm the ACT exp table set during P1
            act_scratch = p1w.tile([1, 8], F32, tag="actw")
            nc.scalar.activation(
                act_scratch, bqk_sb[0:1, 0:8],
                mybir.ActivationFunctionType.Exp,
            )

            # ---- P1 chunk emitters ----
            def emit_chunk_dma(sc):
                chunk_x[sc] = p1sb.tile([128, DT, CB], BF16, tag="xts",
                                        name="xts")
                nc.sync.dma_start(out=chunk_x[sc], in_=x_chunk_ap(sc))

            def emit_qk_group(sc, et):
                xts = chunk_x[sc]
                pqk = psC.tile([128, EC], F32, tag="C", name="pqk")[:, 0:CB]
                for dt in range(DT):
                    nc.tensor.matmul(
                        pqk,
                        w_sb[:, dt, et * 128:(et + 1) * 128],
                        xts[:, dt, :],
                        start=(dt == 0),
                        stop=(dt == DT - 1),
                    )
                dst = qT if et < NP else kT
                slab = et if et < NP else et - NP
                nc.vector.tensor_scalar(
                    out=dst[:, slab, sc * CB:sc * CB + CB],
                    in0=pqk,
                    scalar1=bqk_sb[:, et:et + 1],
                    scalar2=scale if et < NP else 1.0,
                    op0=mybir.AluOpType.add,
                    op1=mybir.AluOpType.mult,
                )

            def emit_v_group(sc, st):
                xts = chunk_x[sc]
                pv = psC.tile([128, EC], F32, tag="C", name="pv")
                for dt in range(DT):
                    nc.tensor.matmul(
                        pv,
                        xts[:, dt, st * 128:(st + 1) * 128],
                        w_sb[:, dt, 2 * EC:3 * EC],
                        start=(dt == 0),
                        stop=(dt == DT - 1),
                    )
                nc.vector.tensor_tensor(
                    out=va[:, sc * (CB // 128) + st, :, 64:128],
                    in0=pv.rearrange("p (h e) -> p h e", e=64),
                    in1=bv_sb.rearrange("p (h e) -> p h e", e=64),
                    op=mybir.AluOpType.add,
                )

            # chunks 0-1 inline: P2 block 0 needs them
            for sc in (0, 1):
                for et in range(n_qk_et):
                    emit_qk_group(sc, et)
                for st in range(CB // 128):
                    emit_v_group(sc, st)
            # ones half of augmented v via on-chip broadcast (no DMA)
            ones64 = p1w.tile([128, 64], F32, tag="ones64")
            nc.vector.memset(ones64, 1.0)
            ones_b = bass.AP(
                tensor=ones64.tensor,
                offset=ones64.offset,
                ap=[ones64.ap[0], [0, HPC], ones64.ap[1]],
            )
            for t in range(N_KT):
                nc.vector.tensor_scalar(
                    out=va[:, t, :, 0:64],
                    in0=ones_b,
                    scalar1=1.0,
                    scalar2=None,
                    op0=mybir.AluOpType.mult,
                )

            # ---- filler deque: chunks 2-7 + P3 groups, emitted between
            # P2 score/av tiles so the PE never idles on ACT ----
            chunk_q = deque()
            p3_q = deque()
            chunk_emitted = [0]
            p3_emitted = [0]
            turn = [0]

            def pop_filler(n):
                for _ in range(n):
                    a, b_, ca, cb = ((chunk_q, p3_q, chunk_emitted, p3_emitted)
                                     if turn[0] % 2 == 0 else
                                     (p3_q, chunk_q, p3_emitted, chunk_emitted))
                    turn[0] += 1
                    if a:
                        a.popleft()()
                        ca[0] += 1
                    elif b_:
                        b_.popleft()()
                        cb[0] += 1

            def drain_chunks_to(target):
                while chunk_emitted[0] < target and chunk_q:
                    chunk_q.popleft()()
                    chunk_emitted[0] += 1

            def drain_p3_to(target):
                while p3_emitted[0] < target and p3_q:
                    p3_q.popleft()()
                    p3_emitted[0] += 1

            from functools import partial
            for sc in range(2, NCH):
                chunk_q.append(partial(emit_chunk_dma, sc))
                for et in range(n_qk_et):
                    chunk_q.append(partial(emit_qk_group, sc, et))
                for st in range(CB // 128):
                    chunk_q.append(partial(emit_v_group, sc, st))
            n_chunk_thunks = len(chunk_q)

            # ---- P2: attention, with interleaved filler ----
            tri_b = bass.AP(
                tensor=tri_sb.tensor,
                offset=tri_sb.offset,
                ap=[tri_sb.ap[0], [0, 2], tri_sb.ap[1]],
            )
            LOOK = 3  # score/exp tiles emitted ahead of attn@v

            def emit_p3_group(st, dh, ysb):
                py = psC.tile([128, EC], F32, tag="C", name="py")[:, 0:QB]
                for eo in range(EC // 128):
                    nc.tensor.matmul(
                        py,
                        aoT[:, eo, st * 128:(st + 1) * 128],
                        wp_sb[:, eo, dh * QB:(dh + 1) * QB],
                        start=(eo == 0),
                        stop=(eo == EC // 128 - 1),
                    )
                nc.vector.tensor_tensor(
                    out=ysb[:, dh * QB:(dh + 1) * QB],
                    in0=py,
                    in1=bp_sb[:, dh * QB:(dh + 1) * QB],
                    op=mybir.AluOpType.add,
                )
                if dh == D // QB - 1:
                    nc.sync.dma_start(
                        out=y.ap()[st * 128:(st + 1) * 128, :],
                        in_=ysb,
                    )

            p3_snapshots = []  # queued-count after each block's p3 append
            for g in range(N_QB):
                q0 = g * QB
                n_t = 4 * (g + 1) if causal else N_KT
                # deadline: this block's scores need chunks <= 2g+1 (all
                # chunks when non-causal)
                if causal:
                    drain_chunks_to(11 * 2 * g)
                else:
                    drain_chunks_to(n_chunk_thunks)
                for p in range(NP):
                    OA = psO.tile([128, QB], F32, tag="OA")
                    OB = psO.tile([128, QB], F32, tag="OB")

                    def emit_score_exp(t):
                        j = t - 4 * g if causal else -1
                        qlo = 128 * j if j >= 0 else 0
                        SAB = psS.tile([128, 2, QB], F32, tag="SAB")
                        k0 = t * KT
                        nc.tensor.matmul(
                            SAB[:, 0, qlo:],
                            kT[0:64, p, k0:k0 + KT],
                            qT[0:64, p, q0 + qlo:q0 + QB],
                            start=True, stop=True,
                        )
                        nc.tensor.matmul(
                            SAB[:, 1, qlo:],
                            kT[64:128, p, k0:k0 + KT],
                            qT[64:128, p, q0 + qlo:q0 + QB],
                            start=True, stop=True,
                        )
                        eAB = p2e.tile([128, 2, QB], BF16, tag="eAB")
                        nc.scalar.activation(
                            eAB[:, :, qlo:], SAB[:, :, qlo:],
                            mybir.ActivationFunctionType.Exp,
                        )
                        if j >= 0:
                            nc.vector.tensor_tensor(
                                out=eAB[:, :, qlo:qlo + 128],
                                in0=eAB[:, :, qlo:qlo + 128],
                                in1=tri_b,
                                op=mybir.AluOpType.mult,
                            )
                        return qlo, eAB

                    def emit_av(t, qlo, eAB):
                        nc.tensor.matmul(
                            OA[:, qlo:],
                            va[:, t, 2 * p, :],
                            eAB[:, 0, qlo:],
                            start=(t == 0), stop=(t == n_t - 1),
                        )
                        nc.tensor.matmul(
                            OB[:, qlo:],
                            va[:, t, 2 * p + 1, :],
                            eAB[:, 1, qlo:],
                            start=(t == 0), stop=(t == n_t - 1),
                        )

                    pending = []
                    for t in range(n_t):
                        pending.append((t, *emit_score_exp(t)))
                        pop_filler(1)
                        if len(pending) > LOOK:
                            emit_av(*pending.pop(0))
                    for item in pending:
                        emit_av(*item)

                    # PE filler while DVE normalizes
                    pop_filler(3)

                    rcpA = p2r.tile([64, QB], F32, tag="rcpA")
                    rcpB = p2r.tile([64, QB], F32, tag="rcpB")
                    nc.vector.reciprocal_approx_fast(out=rcpA, in_=OA[0:64, :])
                    nc.vector.reciprocal_approx_fast(out=rcpB, in_=OB[0:64, :])
                    nc.vector.tensor_tensor(
                        out=aoT[0:64, p, q0:q0 + QB],
                        in0=OA[64:128, :],
                        in1=rcpA,
                        op=mybir.AluOpType.mult,
                    )
                    nc.vector.tensor_tensor(
                        out=aoT[64:128, p, q0:q0 + QB],
                        in0=OB[64:128, :],
                        in1=rcpB,
                        op=mybir.AluOpType.mult,
                    )

                # before reusing ysb buffers (bufs=2), the previous block's
                # p3 thunks (their readers) must all be emitted
                if len(p3_snapshots) >= 1:
                    drain_p3_to(p3_snapshots[-1])
                for st in range(4 * g, 4 * (g + 1)):
                    ysb = p3y.tile([128, D], F32, tag="ysb", name="ysb")
                    for dh in range(D // QB):
                        p3_q.append(partial(emit_p3_group, st, dh, ysb))
                p3_snapshots.append(p3_snapshots[-1] + 8 if p3_snapshots
                                    else 8)
            while chunk_q or p3_q:
                pop_filler(1)

    nc.compile()
    return nc


def _get_nc(causal: bool):
    if causal not in _nc_cache:
        _nc_cache[causal] = _build(causal)
    return _nc_cache[causal]


def _numpy_fallback(x, mask, w_attn, b_attn, w_proj, b_proj):
    x64 = x.astype(np.float64)
    qkv = x64 @ w_attn.astype(np.float64) + b_attn.astype(np.float64)
    q, k, v = np.split(qkv, 3, axis=-1)
    sp = lambda t: t.reshape(B, S, H, HD).transpose(0, 2, 1, 3)
    q, k, v = sp(q), sp(k), sp(v)
    scores = np.einsum("bhqd,bhkd->bhqk", q, k) / math.sqrt(HD)
    m = np.broadcast_to(np.asarray(mask, bool), scores.shape)
    scores = np.where(m, scores, -np.inf)
    scores -= scores.max(axis=-1, keepdims=True)
    e = np.exp(scores)
    attn = e / e.sum(axis=-1, keepdims=True)
    out = np.einsum("bhqk,bhkd->bhqd", attn, v)
    out = out.transpose(0, 2, 1, 3).reshape(B, S, D)
    return (out @ w_proj.astype(np.float64) + b_proj.astype(np.float64)).astype(
        np.float32
    )


def kernel(x, mask, w_attn, b_attn, w_proj, b_proj) -> np.ndarray:
    from concourse.bass_utils import run_bass_kernel_spmd

    x = np.asarray(x, dtype=np.float32)
    w_attn = np.asarray(w_attn, dtype=np.float32)
    b_attn = np.asarray(b_attn, dtype=np.float32)
    w_proj = np.asarray(w_proj, dtype=np.float32)
    b_proj = np.asarray(b_proj, dtype=np.float32)

    m2 = np.asarray(mask, dtype=bool).reshape(S, S)
    if np.array_equal(m2, np.tril(np.ones((S, S), dtype=bool))):
        causal = True
    elif m2.all():
        causal = False
    else:
        return _numpy_fallback(x, mask, w_attn, b_attn, w_proj, b_proj)

    nc = _get_nc(causal)

    import ml_dtypes
    BF = ml_dtypes.bfloat16
    tri_np = np.triu(np.ones((128, 128), dtype=BF))

    in_maps = []
    for c in range(8):
        b, hg = divmod(c, 2)
        e0 = hg * EC
        q_sl = slice(e0, e0 + EC)
        k_sl = slice(D + e0, D + e0 + EC)
        v_sl = slice(2 * D + e0, 2 * D + e0 + EC)
        wq = w_attn[:, q_sl]
        wk = w_attn[:, k_sl]
        wv = w_attn[:, v_sl]
        # device evac computes (q_psum + bias) * scale for q tiles, so the
        # raw biases are passed
        bqk_np = np.concatenate([b_attn[q_sl], b_attn[k_sl]]).reshape(
            2 * EC // 128, 128).T
        in_maps.append({
            "xT": np.ascontiguousarray(x[b].T).astype(BF),
            "wqkv": np.concatenate([wq, wk, wv], axis=1).astype(BF),
            "bqk": np.ascontiguousarray(bqk_np, dtype=np.float32),
            "bv": b_attn[v_sl].reshape(1, EC).copy(),
            "wp": np.ascontiguousarray(w_proj[q_sl, :]).astype(BF),
            "bp": (b_proj if hg == 0 else np.zeros_like(b_proj)).reshape(1, D).copy(),
            "tri": tri_np,
        })

    trace = os.environ.get("KERNEL_TRACE") == "1"
    res = run_bass_kernel_spmd(nc, in_maps, core_ids=list(range(8)), trace=trace)
    global last_exec_time_ns
    if res.exec_time_ns is not None:
        last_exec_time_ns = res.exec_time_ns
    parts = [res.results[c]["y"] for c in range(8)]
    out = np.empty((B, S, D), dtype=np.float32)
    for b in range(B):
        out[b] = parts[2 * b] + parts[2 * b + 1]
    return out



# revision 26
# speedup vs baseline: 1.0073x; 1.0073x over previous
"""Trainium2 Bass kernel for causal multi-head attention.

Problem: B=4, S=2048, D=1024, H=16 (head_dim 64), fp32.
  qkv = x @ w_attn + b_attn ; causal SDPA ; out @ w_proj + b_proj

Sharding (8 cores): data-parallel over B (4) x tensor-parallel over head
halves (2). Core c handles batch b=c//2, heads [8*(c%2), 8*(c%2)+8).
Each core computes its qkv slice, its heads' attention, and a partial
output projection (its heads' rows of w_proj); the host sums the two
partials per batch. b_proj is added on even cores (odd cores get zeros).

Device dataflow (per core):
  P1: qT,kT = (w_q|k)^T @ x^T  -> [e, s] layout (e on partitions);
      v = x @ w_v (natural [s, e]); biases + 1/sqrt(hd) fused into the
      PSUM->SBUF evacuations. v is stored augmented as [v_h | ones].
  P2: per head-pair p, query block g (512 wide), key tile t (128):
      scoresT = kT.T @ qT (2 heads row-tiled), exp on ACT (no max
      subtraction: scores are O(+-6)), block-causal via narrowed moving
      ranges + a triangular multiply on diagonal tiles, then
      [outT | den] += [v | ones].T @ expT  (fp32r requires col_grp=0xf,
      so den rides along in the same M=128 matmul, replicated over
      partitions 64:128). Normalization by 1/den applies to outT only.
  P3: y_partial = aoT.T @ w_proj_slice (+ b_proj), DMA out.

All matmuls run as float32r (E8M11 weights, full PE rate at moving
dim >= 256; measured matmul norm-rel error ~1e-7 on rounded inputs).
"""

import math
import os

import numpy as np

import concourse.bass as bass
import concourse.mybir as mybir
import concourse.tile as tile
from concourse import bacc

last_exec_time_ns = None

B, S, D, H = 4, 2048, 1024, 16
HD = D // H          # 64
HPC = H // 2         # heads per core = 8
EC = HPC * HD        # per-core qkv slice width = 512
NP = 4               # head pairs per core
QB = 512             # query block width
KT = 128             # key tile
N_QB = S // QB       # 4
N_KT = S // KT       # 16
DT = D // 128        # 8 contraction tiles

F32 = mybir.dt.float32
BF16 = mybir.dt.bfloat16

_nc_cache: dict = {}


def _round_fp32r(x: np.ndarray) -> np.ndarray:
    """Round-to-nearest-even fp32 -> fp32r (E8M11: low 12 mantissa bits 0)."""
    u = np.ascontiguousarray(x, dtype=np.float32).view(np.uint32)
    u2 = (u + 0x7FF + ((u >> 12) & 1)) & np.uint32(0xFFFFF000)
    return u2.view(np.float32)


def _build(causal: bool):
    nc = bacc.Bacc("TRN2", target_bir_lowering=False)
    xT = nc.dram_tensor("xT", [D, S], BF16, kind="ExternalInput")
    wqkv = nc.dram_tensor("wqkv", [D, 3 * EC], BF16, kind="ExternalInput")
    bqk = nc.dram_tensor("bqk", [128, 2 * EC // 128], F32, kind="ExternalInput")
    bv = nc.dram_tensor("bv", [1, EC], F32, kind="ExternalInput")
    wp = nc.dram_tensor("wp", [EC, D], BF16, kind="ExternalInput")
    bp = nc.dram_tensor("bp", [1, D], F32, kind="ExternalInput")
    tri = nc.dram_tensor("tri", [128, 128], BF16, kind="ExternalInput")
    y = nc.dram_tensor("y", [S, D], F32, kind="ExternalOutput")

    n_qk_et = 2 * EC // 128   # 8 e-tiles for q+k
    scale = 1.0 / math.sqrt(HD)
    CB = 512                  # P1 s-chunk width
    NCH = S // CB

    from collections import deque

    with tile.TileContext(nc) as tc:
        with (
            tc.tile_pool(name="consts", bufs=1) as consts,
            tc.tile_pool(name="qkvp", bufs=1) as qkvp,
            tc.tile_pool(name="p1sb", bufs=2) as p1sb,
            tc.tile_pool(name="p1w", bufs=1) as p1w,
            tc.tile_pool(name="p3w", bufs=1) as p3w,
            tc.tile_pool(name="p2e", bufs=4) as p2e,
            tc.tile_pool(name="p2r", bufs=1) as p2r,
            tc.tile_pool(name="p3y", bufs=2) as p3y,
            tc.tile_pool(name="psS", bufs=2, space="PSUM") as psS,
            tc.tile_pool(name="psO", bufs=1, space="PSUM") as psO,
            tc.tile_pool(name="psC", bufs=2, space="PSUM") as psC,
        ):
            bqk_sb = consts.tile([128, n_qk_et], F32, tag="bqk_sb")
            tri_sb = consts.tile([128, 128], BF16, tag="tri_sb")
            qT = qkvp.tile([128, NP, S], BF16, tag="qT")
            kT = qkvp.tile([128, NP, S], BF16, tag="kT")
            # augmented v: per head h and key tile t, [ones | v_h] so a
            # single M=128 matmul yields both attn@v and the softmax
            # denominator (on out partitions 0:64)
            va = qkvp.tile([128, N_KT, HPC, 128], BF16, tag="va")
            aoT = qkvp.tile([128, NP, S], BF16, tag="aoT")
            bv_sb = p1w.tile([128, EC], F32, tag="bv_sb")
            w_sb = p1w.tile([128, DT, 3 * EC], BF16, tag="w_sb")
            bp_sb = p3w.tile([128, D], F32, tag="bp_sb")
            wp_sb = p3w.tile([128, EC // 128, D], BF16, tag="wp_sb")

            def x_chunk_ap(sc):
                # one 3D DMA per chunk: [128, DT, CB] from xT[d, s]:
                # elem (p, dt, c) <- xT[dt*128+p, sc*CB+c]
                a = xT.ap()
                return bass.AP(
                    tensor=a.tensor,
                    offset=a.offset + sc * CB,
                    ap=[[S, 128], [128 * S, DT], [1, CB]],
                )

            # ---- prologue DMAs: x chunks 0-1 + biases first on sync,
            # weight slabs in dt order alternating scalar/gpsimd ----
            chunk_x = {}
            chunk_x[0] = p1sb.tile([128, DT, CB], BF16, tag="xts",
                                   name="xts")
            nc.sync.dma_start(out=chunk_x[0], in_=x_chunk_ap(0))
            nc.sync.dma_start(out=bqk_sb, in_=bqk.ap())
            nc.sync.dma_start(out=bv_sb, in_=bv.ap().to_broadcast([128, EC]))
            HW_ = 3 * EC // 2
            for half in range(2):
                for dt in range(DT):
                    if half == 0 and dt >= 6:
                        eng = nc.sync
                    else:
                        eng = nc.scalar if dt % 2 == 0 else nc.gpsimd
                    eng.dma_start(
                        out=w_sb[:, dt, half * HW_:(half + 1) * HW_],
                        in_=wqkv.ap()[dt * 128:(dt + 1) * 128,
                                      half * HW_:(half + 1) * HW_],
                    )
            nc.gpsimd.dma_start(out=tri_sb, in_=tri.ap())
            # P3 constants prefetched behind the x chunks on sync
            nc.sync.dma_start(out=bp_sb, in_=bp.ap().to_broadcast([128, D]))
            for eo in range(EC // 128):
                nc.sync.dma_start(
                    out=wp_sb[:, eo, :],
                    in_=wp.ap()[eo * 128:(eo + 1) * 128, :],
                )
            # warm the ACT exp table set during P1
            act_scratch = p1w.tile([1, 8], F32, tag="actw")
            nc.scalar.activation(
                act_scratch, bqk_sb[0:1, 0:8],
                mybir.ActivationFunctionType.Exp,
            )

            # ---- P1 chunk emitters ----
            def emit_chunk_dma(sc):
                chunk_x[sc] = p1sb.tile([128, DT, CB], BF16, tag="xts",
                                        name="xts")
                nc.sync.dma_start(out=chunk_x[sc], in_=x_chunk_ap(sc))

            def emit_qk_group(sc, et):
                xts = chunk_x[sc]
                pqk = psC.tile([128, EC], F32, tag="C", name="pqk")
                for dt in range(DT):
                    nc.tensor.matmul(
                        pqk,
                        w_sb[:, dt, et * 128:(et + 1) * 128],
                        xts[:, dt, :],
                        start=(dt == 0),
                        stop=(dt == DT - 1),
                    )
                dst = qT if et < NP else kT
                slab = et if et < NP else et - NP
                nc.vector.tensor_scalar(
                    out=dst[:, slab, sc * CB:sc * CB + CB],
                    in0=pqk,
                    scalar1=bqk_sb[:, et:et + 1],
                    scalar2=scale if et < NP else 1.0,
                    op0=mybir.AluOpType.add,
                    op1=mybir.AluOpType.mult,
                )

            def emit_v_group(sc, st):
                xts = chunk_x[sc]
                pv = psC.tile([128, EC], F32, tag="C", name="pv")
                for dt in range(DT):
                    nc.tensor.matmul(
                        pv,
                        xts[:, dt, st * 128:(st + 1) * 128],
                        w_sb[:, dt, 2 * EC:3 * EC],
                        start=(dt == 0),
                        stop=(dt == DT - 1),
                    )
                nc.vector.tensor_tensor(
                    out=va[:, sc * (CB // 128) + st, :, 64:128],
                    in0=pv.rearrange("p (h e) -> p h e", e=64),
                    in1=bv_sb.rearrange("p (h e) -> p h e", e=64),
                    op=mybir.AluOpType.add,
                )

            # chunk 0 inline: P2 block 0 needs it
            for sc in (0,):
                for et in range(n_qk_et):
                    emit_qk_group(sc, et)
                for st in range(CB // 128):
                    emit_v_group(sc, st)
            # ones half of augmented v via on-chip broadcast (no DMA)
            ones64 = p1w.tile([128, 64], F32, tag="ones64")
            nc.vector.memset(ones64, 1.0)
            ones_b = bass.AP(
                tensor=ones64.tensor,
                offset=ones64.offset,
                ap=[ones64.ap[0], [0, HPC], ones64.ap[1]],
            )
            for t in range(N_KT):
                nc.vector.tensor_scalar(
                    out=va[:, t, :, 0:64],
                    in0=ones_b,
                    scalar1=1.0,
                    scalar2=None,
                    op0=mybir.AluOpType.mult,
                )

            # ---- filler deque: chunks 2-7 + P3 groups, emitted between
            # P2 score/av tiles so the PE never idles on ACT ----
            chunk_q = deque()
            p3_q = deque()
            chunk_emitted = [0]
            p3_emitted = [0]
            turn = [0]

            def pop_filler(n):
                for _ in range(n):
                    a, b_, ca, cb = ((chunk_q, p3_q, chunk_emitted, p3_emitted)
                                     if turn[0] % 2 == 0 else
                                     (p3_q, chunk_q, p3_emitted, chunk_emitted))
                    turn[0] += 1
                    if a:
                        a.popleft()()
                        ca[0] += 1
                    elif b_:
                        b_.popleft()()
                        cb[0] += 1

            def drain_chunks_to(target):
                while chunk_emitted[0] < target and chunk_q:
                    chunk_q.popleft()()
                    chunk_emitted[0] += 1

            def drain_p3_to(target):
                while p3_emitted[0] < target and p3_q:
                    p3_q.popleft()()
                    p3_emitted[0] += 1

            from functools import partial
            for sc in range(1, NCH):
                chunk_q.append(partial(emit_chunk_dma, sc))
                for et in range(n_qk_et):
                    chunk_q.append(partial(emit_qk_group, sc, et))
                for st in range(CB // 128):
                    chunk_q.append(partial(emit_v_group, sc, st))
            n_chunk_thunks = len(chunk_q)

            # ---- P2: attention, with interleaved filler ----
            tri_b = bass.AP(
                tensor=tri_sb.tensor,
                offset=tri_sb.offset,
                ap=[tri_sb.ap[0], [0, 2], tri_sb.ap[1]],
            )
            LOOK = 3  # score/exp tiles emitted ahead of attn@v

            def emit_p3_group(st, dh, ysb):
                py = psC.tile([128, EC], F32, tag="C", name="py")[:, 0:QB]
                for eo in range(EC // 128):
                    nc.tensor.matmul(
                        py,
                        aoT[:, eo, st * 128:(st + 1) * 128],
                        wp_sb[:, eo, dh * QB:(dh + 1) * QB],
                        start=(eo == 0),
                        stop=(eo == EC // 128 - 1),
                    )
                nc.vector.tensor_tensor(
                    out=ysb[:, dh * QB:(dh + 1) * QB],
                    in0=py,
                    in1=bp_sb[:, dh * QB:(dh + 1) * QB],
                    op=mybir.AluOpType.add,
                )
                if dh == D // QB - 1:
                    nc.sync.dma_start(
                        out=y.ap()[st * 128:(st + 1) * 128, :],
                        in_=ysb,
                    )

            p3_snapshots = []  # queued-count after each block's p3 append
            for g in range(N_QB):
                q0 = g * QB
                n_t = 4 * (g + 1) if causal else N_KT
                # deadline: this block's scores need chunks <= 2g+1 (all
                # chunks when non-causal)
                if causal:
                    drain_chunks_to(13 * g)
                else:
                    drain_chunks_to(n_chunk_thunks)
                for p in range(NP):
                    OA = psO.tile([128, QB], F32, tag="OA")
                    OB = psO.tile([128, QB], F32, tag="OB")

                    def emit_score_exp(t):
                        j = t - 4 * g if causal else -1
                        qlo = 128 * j if j >= 0 else 0
                        SAB = psS.tile([128, 2, QB], F32, tag="SAB")
                        k0 = t * KT
                        nc.tensor.matmul(
                            SAB[:, 0, qlo:],
                            kT[0:64, p, k0:k0 + KT],
                            qT[0:64, p, q0 + qlo:q0 + QB],
                            start=True, stop=True,
                        )
                        nc.tensor.matmul(
                            SAB[:, 1, qlo:],
                            kT[64:128, p, k0:k0 + KT],
                            qT[64:128, p, q0 + qlo:q0 + QB],
                            start=True, stop=True,
                        )
                        eAB = p2e.tile([128, 2, QB], BF16, tag="eAB")
                        nc.scalar.activation(
                            eAB[:, :, qlo:], SAB[:, :, qlo:],
                            mybir.ActivationFunctionType.Exp,
                        )
                        if j >= 0:
                            nc.vector.tensor_tensor(
                                out=eAB[:, :, qlo:qlo + 128],
                                in0=eAB[:, :, qlo:qlo + 128],
                                in1=tri_b,
                                op=mybir.AluOpType.mult,
                            )
                        return qlo, eAB

                    def emit_av(t, qlo, eAB):
                        nc.tensor.matmul(
                            OA[:, qlo:],
                            va[:, t, 2 * p, :],
                            eAB[:, 0, qlo:],
                            start=(t == 0), stop=(t == n_t - 1),
                        )
                        nc.tensor.matmul(
                            OB[:, qlo:],
                            va[:, t, 2 * p + 1, :],
                            eAB[:, 1, qlo:],
                            start=(t == 0), stop=(t == n_t - 1),
                        )

                    pending = []
                    for t in range(n_t):
                        pending.append((t, *emit_score_exp(t)))
                        pop_filler(1)
                        if len(pending) > LOOK:
                            emit_av(*pending.pop(0))
                    for item in pending:
                        emit_av(*item)

                    # PE filler while DVE normalizes
                    pop_filler(3)

                    rcpA = p2r.tile([64, QB], F32, tag="rcpA")
                    rcpB = p2r.tile([64, QB], F32, tag="rcpB")
                    nc.vector.reciprocal_approx_fast(out=rcpA, in_=OA[0:64, :])
                    nc.vector.reciprocal_approx_fast(out=rcpB, in_=OB[0:64, :])
                    nc.vector.tensor_tensor(
                        out=aoT[0:64, p, q0:q0 + QB],
                        in0=OA[64:128, :],
                        in1=rcpA,
                        op=mybir.AluOpType.mult,
                    )
                    nc.vector.tensor_tensor(
                        out=aoT[64:128, p, q0:q0 + QB],
                        in0=OB[64:128, :],
                        in1=rcpB,
                        op=mybir.AluOpType.mult,
                    )

                # before reusing ysb buffers (bufs=2), the previous block's
                # p3 thunks (their readers) must all be emitted
                if len(p3_snapshots) >= 1:
                    drain_p3_to(p3_snapshots[-1])
                for st in range(4 * g, 4 * (g + 1)):
                    ysb = p3y.tile([128, D], F32, tag="ysb", name="ysb")
                    for dh in range(D // QB):
                        p3_q.append(partial(emit_p3_group, st, dh, ysb))
                p3_snapshots.append(p3_snapshots[-1] + 8 if p3_snapshots
                                    else 8)
            while chunk_q or p3_q:
                pop_filler(1)

    nc.compile()
    return nc


def _get_nc(causal: bool):
    if causal not in _nc_cache:
        _nc_cache[causal] = _build(causal)
    return _nc_cache[causal]


def _numpy_fallback(x, mask, w_attn, b_attn, w_proj, b_proj):
    x64 = x.astype(np.float64)
    qkv = x64 @ w_attn.astype(np.float64) + b_attn.astype(np.float64)
    q, k, v = np.split(qkv, 3, axis=-1)
    sp = lambda t: t.reshape(B, S, H, HD).transpose(0, 2, 1, 3)
    q, k, v = sp(q), sp(k), sp(v)
    scores = np.einsum("bhqd,bhkd->bhqk", q, k) / math.sqrt(HD)
    m = np.broadcast_to(np.asarray(mask, bool), scores.shape)
    scores = np.where(m, scores, -np.inf)
    scores -= scores.max(axis=-1, keepdims=True)
    e = np.exp(scores)
    attn = e / e.sum(axis=-1, keepdims=True)
    out = np.einsum("bhqk,bhkd->bhqd", attn, v)
    out = out.transpose(0, 2, 1, 3).reshape(B, S, D)
    return (out @ w_proj.astype(np.float64) + b_proj.astype(np.float64)).astype(
        np.float32
    )


def kernel(x, mask, w_attn, b_attn, w_proj, b_proj) -> np.ndarray:
    from concourse.bass_utils import run_bass_kernel_spmd

    x = np.asarray(x, dtype=np.float32)
    w_attn = np.asarray(w_attn, dtype=np.float32)
    b_attn = np.asarray(b_attn, dtype=np.float32)
    w_proj = np.asarray(w_proj, dtype=np.float32)
    b_proj = np.asarray(b_proj, dtype=np.float32)

    m2 = np.asarray(mask, dtype=bool).reshape(S, S)
    if np.array_equal(m2, np.tril(np.ones((S, S), dtype=bool))):
        causal = True
    elif m2.all():
        causal = False
    else:
        return _numpy_fallback(x, mask, w_attn, b_attn, w_proj, b_proj)

    nc = _get_nc(causal)

    import ml_dtypes
    BF = ml_dtypes.bfloat16
    tri_np = np.triu(np.ones((128, 128), dtype=BF))

    in_maps = []
    for c in range(8):
        b, hg = divmod(c, 2)
        e0 = hg * EC
        q_sl = slice(e0, e0 + EC)
        k_sl = slice(D + e0, D + e0 + EC)
        v_sl = slice(2 * D + e0, 2 * D + e0 + EC)
        wq = w_attn[:, q_sl]
        wk = w_attn[:, k_sl]
        wv = w_attn[:, v_sl]
        # device evac computes (q_psum + bias) * scale for q tiles, so the
        # raw biases are passed
        bqk_np = np.concatenate([b_attn[q_sl], b_attn[k_sl]]).reshape(
            2 * EC // 128, 128).T
        in_maps.append({
            "xT": np.ascontiguousarray(x[b].T).astype(BF),
            "wqkv": np.concatenate([wq, wk, wv], axis=1).astype(BF),
            "bqk": np.ascontiguousarray(bqk_np, dtype=np.float32),
            "bv": b_attn[v_sl].reshape(1, EC).copy(),
            "wp": np.ascontiguousarray(w_proj[q_sl, :]).astype(BF),
            "bp": (b_proj if hg == 0 else np.zeros_like(b_proj)).reshape(1, D).copy(),
            "tri": tri_np,
        })

    trace = os.environ.get("KERNEL_TRACE") == "1"
    res = run_bass_kernel_spmd(nc, in_maps, core_ids=list(range(8)), trace=trace)
    global last_exec_time_ns
    if res.exec_time_ns is not None:
        last_exec_time_ns = res.exec_time_ns
    parts = [res.results[c]["y"] for c in range(8)]
    out = np.empty((B, S, D), dtype=np.float32)
    for b in range(B):
        out[b] = parts[2 * b] + parts[2 * b + 1]
    return out

